# revision 1
# baseline (speedup 1.0000x reference)
"""Trainium2 Bass kernel for CrossNetGatingMixLayer.

Math (per layer i, with U,C,V per expert e; gate = softmax over a singleton
axis == 1.0 identically, so the gating einsum and G are dead code):

    xv = tanh(xl @ V[e])          (B,R)  per expert
    xc = tanh(xv @ C[e].T)        (B,R)
    xu = xc @ U[e].T              (B,D)
    xl = xl + x0 * (sum_e xu + E * bias)

Strategy: data-parallel over 8 NeuronCores (batch split 16384 -> 8 x 2048).
On-chip layout is transposed ([d, b]): all matmuls contract over d or r with
the contraction dim on SBUF partitions.  Matmuls run in float32r (4x faster
than fp32 on the PE; inputs rounded to 11 mantissa bits) while the residual
stream xl stays fp32.  x is transposed in/out via PE-transpose, batched in
groups of four 128x128 blocks per PSUM tile so eviction copies are wide.
"""
import numpy as np
from contextlib import ExitStack

import concourse.bass as bass
from concourse import bacc
import concourse.mybir as mybir
import concourse.tile as tile
from concourse.bass_utils import run_bass_kernel_spmd
from concourse.masks import make_identity

B, D, L, E, R = 16384, 512, 3, 4, 128
NCORES = 8
BL = B // NCORES            # 2048 rows per core
NBT = BL // 128             # 16 batch tiles of 128
NBC = BL // 512             # 4 batch chunks of 512 (matmul free dim)
ND = D // 128               # 4 d-chunks of 128
f32 = mybir.dt.float32
f32r = mybir.dt.float32r
Tanh = mybir.ActivationFunctionType.Tanh

_prog_cache = {}


def _build(has_bias: bool, use_f32r: bool):
    mmdt = f32r if use_f32r else f32
    nc = bacc.Bacc("TRN2")
    x_d = nc.declare_dram_parameter("x", [BL, D], f32, isOutput=False)
    Vs_d = nc.declare_dram_parameter("Vs", [L, E, D, R], f32, isOutput=False)
    Cs_d = nc.declare_dram_parameter("Cs", [L, E, R, R], f32, isOutput=False)
    Us_d = nc.declare_dram_parameter("Us", [L, E, D, R], f32, isOutput=False)
    if has_bias:
        b_d = nc.declare_dram_parameter("b", [L, D], f32, isOutput=False)
    out_d = nc.declare_dram_parameter("out", [BL, D], f32, isOutput=True)

    with tile.TileContext(nc) as tc, ExitStack() as ctx:
        const = ctx.enter_context(tc.tile_pool(name="const", bufs=1))
        wpool = ctx.enter_context(tc.tile_pool(name="wpool", bufs=1))
        xpool = ctx.enter_context(tc.tile_pool(name="xpool", bufs=1))
        wtmp_p = ctx.enter_context(tc.tile_pool(name="wtmp_p", bufs=2))
        ptr = ctx.enter_context(tc.tile_pool(name="ptr", bufs=2, space="PSUM"))
        ph_p = ctx.enter_context(tc.tile_pool(name="ph_p", bufs=3, space="PSUM"))
        pz_p = ctx.enter_context(tc.tile_pool(name="pz_p", bufs=1, space="PSUM"))
        pu_p = ctx.enter_context(tc.tile_pool(name="pu_p", bufs=2, space="PSUM"))

        ident = const.tile([128, 128], f32)
        make_identity(nc, ident)

        # ---- persistent weight tiles (mmdt) ----
        Vr = wpool.tile([128, L, E, ND, R], mmdt)    # V[l,e] kd-chunk: [d128, r128]
        Cr = wpool.tile([128, L, E, R], mmdt)        # C[l,e].T: [s128, r128]
        Ur = wpool.tile([128, L, E, ND, 128], mmdt)  # U[l,e].T kd-chunk: [r128, d128]

        def prep_V(l):
            vtmp = wtmp_p.tile([128, E, ND, R], f32, name=f"vtmp{l}", tag="wtmp")
            nc.gpsimd.dma_start(
                out=vtmp,
                in_=Vs_d[l].rearrange("e (kd p) r -> p e kd r", p=128))
            nc.any.tensor_copy(Vr[:, l], vtmp)

        def prep_U(l):
            # U: [d, r] -> PE transpose to [r, d] chunks, batched 4-wide
            utmp = wtmp_p.tile([128, E, ND, R], f32, name=f"utmp{l}", tag="wtmp")
            nc.gpsimd.dma_start(
                out=utmp,
                in_=Us_d[l].rearrange("e (kd p) r -> p e kd r", p=128))
            for e in range(E):
                put = ptr.tile([128, 512], f32, name=f"put{l}_{e}", tag="tr")
                for kd in range(ND):
                    nc.tensor.transpose(
                        put[:, 128 * kd:128 * (kd + 1)], utmp[:, e, kd, :],
                        ident)
                nc.any.tensor_copy(
                    Ur[:, l, e].rearrange("p a b -> p (a b)"), put)

        def prep_C(l):
            # C: [r, s] -> [s, r], 4 experts batched into one psum tile
            ctmp = wtmp_p.tile([128, E, R], f32, name=f"ctmp{l}", tag="wtmp")
            nc.gpsimd.dma_start(out=ctmp, in_=Cs_d[l].rearrange("e r s -> r e s"))
            pct = ptr.tile([128, 512], f32, name=f"pct{l}", tag="tr")
            for e in range(E):
                nc.tensor.transpose(
                    pct[:, 128 * e:128 * (e + 1)], ctmp[:, e, :], ident)
            nc.any.tensor_copy(Cr[:, l].rearrange("p a b -> p (a b)"), pct)

        if has_bias:
            btmp = wtmp_p.tile([1, L * D], f32, name="btmp", tag="bias", bufs=1)
            nc.sync.dma_start(out=btmp,
                              in_=b_d[:].rearrange("l d -> (l d)")[None, :])
            bias4 = wpool.tile([1, L * D], mmdt)
            nc.scalar.mul(bias4, btmp, float(E))
            ones_t = wtmp_p.tile([1, 512], f32, name="ones_t", tag="ones1", bufs=1)
            nc.vector.memset(ones_t, 1.0)
            ones_r = wpool.tile([1, 512], mmdt)
            nc.vector.tensor_copy(ones_r, ones_t)

        # ---- x: natural load + PE transpose into [d, b] layout ----
        # Order: V(l=0) first so mm1 can start as soon as batch-group g=0 is
        # transposed; group-major transpose order so chunk c only needs the
        # first c+1 groups; x0r copied per group straight from PSUM.
        xlT = xpool.tile([128, ND, BL], f32)      # residual stream, fp32
        x0r = xpool.tile([128, ND, BL], mmdt)     # original x, matmul dtype
        with tc.tile_pool(name="xnat_p", bufs=1) as xnat_p:
            xnat = xnat_p.tile([128, NBT, D], f32)
            # first batch-group arrives in column chunks so the dc=0
            # transposes can start after 256KB instead of 1MB
            for dc in range(ND):
                for t in range(4):
                    nc.sync.dma_start(
                        out=xnat[:, t, 128 * dc:128 * (dc + 1)],
                        in_=x_d[128 * t:128 * (t + 1),
                                128 * dc:128 * (dc + 1)])
                if dc == 0:
                    prep_V(0)
                elif dc == 1:
                    prep_C(0)
            for t in range(4, NBT):
                nc.sync.dma_start(
                    out=xnat[:, t, :],
                    in_=x_d[128 * t:128 * (t + 1), :])
            for g in range(NBT // 4):
                for dc in range(ND):
                    pxt = ptr.tile([128, 512], f32, name=f"pxt{dc}_{g}",
                                   tag="tr")
                    for i in range(4):
                        nc.tensor.transpose(
                            pxt[:, 128 * i:128 * (i + 1)],
                            xnat[:, 4 * g + i, 128 * dc:128 * (dc + 1)],
                            ident)
                    nc.any.tensor_copy(
                        xlT[:, dc, 512 * g:512 * (g + 1)], pxt)
                    nc.any.tensor_copy(
                        x0r[:, dc, 512 * g:512 * (g + 1)], pxt)
                if g == 0:
                    prep_U(0)
                elif g == 1:
                    prep_V(1)
                    prep_C(1)
                elif g == 2:
                    prep_U(1)
                elif g == 3:
                    prep_V(2)
                    prep_C(2)
                    prep_U(2)

        # ---- main layer loop ----
        hz_p = ctx.enter_context(tc.tile_pool(name="hz_p", bufs=1))
        tmp_p = ctx.enter_context(tc.tile_pool(name="tmp_p", bufs=4))
        xlr_p = ctx.enter_context(tc.tile_pool(name="xlr_p", bufs=2))
        onat_p = ctx.enter_context(tc.tile_pool(name="onat_p", bufs=3))

        for l in range(L):
            for c in range(NBC):
                cols = slice(512 * c, 512 * (c + 1))
                if l == 0:
                    rhs1 = x0r
                    rcols = cols
                elif use_f32r:
                    xlr = xlr_p.tile([128, ND, 512], f32r,
                                     name=f"xlr{l}_{c}", tag="xlr")
                    for dc in range(ND):
                        nc.any.tensor_copy(xlr[:, dc, :], xlT[:, dc, cols])
                    rhs1 = xlr
                    rcols = slice(0, 512)
                else:
                    rhs1 = xlT
                    rcols = cols

                zr = []
                for e in range(E):
                    ph = ph_p.tile([128, 512], f32, name=f"ph{l}_{c}_{e}",
                                   tag="ph")
                    for kd in range(ND):
                        nc.tensor.matmul(
                            ph,
                            lhsT=Vr[:, l, e, kd, :],
                            rhs=rhs1[:, kd, rcols],
                            start=(kd == 0), stop=(kd == ND - 1))
                    hr = hz_p.tile([128, 512], mmdt, name=f"h{l}_{c}_{e}",
                                   tag="h", bufs=6)
                    nc.scalar.activation(hr, ph, Tanh)

                    pz = pz_p.tile([128, 512], f32, name=f"pz{l}_{c}_{e}",
                                   tag="pz")
                    nc.tensor.matmul(pz, lhsT=Cr[:, l, e, :], rhs=hr,
                                     start=True, stop=True)
                    z = hz_p.tile([128, 512], mmdt, name=f"z{l}_{c}_{e}",
                                  tag="z", bufs=7 if has_bias else 8)
                    nc.scalar.activation(z, pz, Tanh)
                    zr.append(z)

                for dc in range(ND):
                    pu = pu_p.tile([128, 512], f32, name=f"pu{l}_{c}_{dc}",
                                   tag="pu")
                    for e in range(E):
                        nc.tensor.matmul(
                            pu, lhsT=Ur[:, l, e, dc, :], rhs=zr[e],
                            start=(e == 0),
                            stop=(e == E - 1 and not has_bias))
                    if has_bias:
                        nc.tensor.matmul(
                            pu,
                            lhsT=bias4[:, l * D + 128 * dc:l * D + 128 * (dc + 1)],
                            rhs=ones_r, start=False, stop=True)
                    tmp = tmp_p.tile([128, 512], f32, name=f"tmp{l}_{c}_{dc}",
                                     tag="tmp")
                    nc.vector.tensor_mul(
                        tmp, pu, x0r[:, dc, cols].bitcast(f32))
                    nc.vector.tensor_add(
                        xlT[:, dc, cols], xlT[:, dc, cols], tmp)

                if l == L - 1:
                    # store this chunk: transpose back to natural + DMA out
                    for t in range(4 * c, 4 * (c + 1)):
                        pot = ptr.tile([128, 512], f32, name=f"pot{t}",
                                       tag="tr")
                        for dc in range(ND):
                            nc.tensor.transpose(
                                pot[:, 128 * dc:128 * (dc + 1)],
                                xlT[:, dc, 128 * t:128 * (t + 1)], ident)
                        onat = onat_p.tile([128, D], f32, name=f"onat{t}",
                                           tag="onat")
                        nc.any.tensor_copy(onat, pot)
                        nc.sync.dma_start(
                            out=out_d[128 * t:128 * (t + 1), :], in_=onat)


    nc.finalize()
    return nc


def _get_prog(has_bias: bool, use_f32r: bool = True):
    key = (has_bias, use_f32r)
    if key not in _prog_cache:
        _prog_cache[key] = _build(has_bias, use_f32r)
    return _prog_cache[key]


def _run(inputs, trace=False, use_f32r=True):
    x = np.ascontiguousarray(np.asarray(inputs["x"], dtype=np.float32))
    Us = np.ascontiguousarray(np.asarray(inputs["Us"], dtype=np.float32))
    Cs = np.ascontiguousarray(np.asarray(inputs["Cs"], dtype=np.float32))
    Vs = np.ascontiguousarray(np.asarray(inputs["Vs"], dtype=np.float32))
    b = np.ascontiguousarray(np.asarray(inputs["b"], dtype=np.float32))
    assert x.shape == (B, D), x.shape
    has_bias = bool(np.any(b))
    nc = _get_prog(has_bias, use_f32r)
    shards = np.split(x, NCORES, axis=0)
    in_maps = []
    for i in range(NCORES):
        m = {"x": shards[i], "Us": Us, "Cs": Cs, "Vs": Vs}
        if has_bias:
            m["b"] = b
        in_maps.append(m)
    res = run_bass_kernel_spmd(nc, in_maps, core_ids=list(range(NCORES)),
                               trace=trace)
    out = np.concatenate([res.results[i]["out"] for i in range(NCORES)],
                         axis=0)
    return out, res


def kernel(**inputs) -> np.ndarray:
    out, _ = _run(inputs)
    return out



# revision 4
# speedup vs baseline: 1.0507x; 1.0507x over previous
"""Trainium2 Bass kernel for CrossNetGatingMixLayer.

Math (per layer i, with U,C,V per expert e; gate = softmax over a singleton
axis == 1.0 identically, so the gating einsum and G are dead code):

    xv = tanh(xl @ V[e])          (B,R)  per expert
    xc = tanh(xv @ C[e].T)        (B,R)
    xu = xc @ U[e].T              (B,D)
    xl = xl + x0 * (sum_e xu + E * bias)

Strategy: data-parallel over 8 NeuronCores (batch split 16384 -> 8 x 2048).
On-chip layout is transposed ([d, b]): all matmuls contract over d or r with
the contraction dim on SBUF partitions.  Matmuls run in float32r (4x faster
than fp32 on the PE; inputs rounded to 11 mantissa bits) while the residual
stream xl stays fp32.  x is transposed in/out via PE-transpose, batched in
groups of four 128x128 blocks per PSUM tile so eviction copies are wide.
"""
import numpy as np
from contextlib import ExitStack

import concourse.bass as bass
from concourse import bacc
import concourse.mybir as mybir
import concourse.tile as tile
from concourse.bass_utils import run_bass_kernel_spmd
from concourse.masks import make_identity

B, D, L, E, R = 16384, 512, 3, 4, 128
NCORES = 8
BL = B // NCORES            # 2048 rows per core
NBT = BL // 128             # 16 batch tiles of 128
NBC = BL // 512             # 4 batch chunks of 512 (matmul free dim)
ND = D // 128               # 4 d-chunks of 128
f32 = mybir.dt.float32
f32r = mybir.dt.float32r
fp8 = mybir.dt.float8e4
DRmode = mybir.MatmulPerfMode.DoubleRow
WSCALE = 64.0
Tanh = mybir.ActivationFunctionType.Tanh

_prog_cache = {}


def _build(has_bias: bool, use_f32r: bool):
    mmdt = f32r if use_f32r else f32
    nc = bacc.Bacc("TRN2")
    x_d = nc.declare_dram_parameter("x", [BL, D], f32, isOutput=False)
    Vs_d = nc.declare_dram_parameter("Vs", [L, E, D, R], f32, isOutput=False)
    Cs_d = nc.declare_dram_parameter("Cs", [L, E, R, R], f32, isOutput=False)
    Us_d = nc.declare_dram_parameter("Us", [L, E, D, R], f32, isOutput=False)
    if has_bias:
        b_d = nc.declare_dram_parameter("b", [L, D], f32, isOutput=False)
    out_d = nc.declare_dram_parameter("out", [BL, D], f32, isOutput=True)

    with tile.TileContext(nc) as tc, ExitStack() as ctx:
        const = ctx.enter_context(tc.tile_pool(name="const", bufs=1))
        wpool = ctx.enter_context(tc.tile_pool(name="wpool", bufs=1))
        xpool = ctx.enter_context(tc.tile_pool(name="xpool", bufs=1))
        wtmp_p = ctx.enter_context(tc.tile_pool(name="wtmp_p", bufs=2))
        ptr = ctx.enter_context(tc.tile_pool(name="ptr", bufs=2, space="PSUM"))
        ph_p = ctx.enter_context(tc.tile_pool(name="ph_p", bufs=3, space="PSUM"))
        pz_p = ctx.enter_context(tc.tile_pool(name="pz_p", bufs=1, space="PSUM"))
        pu_p = ctx.enter_context(tc.tile_pool(name="pu_p", bufs=2, space="PSUM"))

        ident = const.tile([128, 128], f32)
        make_identity(nc, ident)

        # ---- persistent weight tiles (mmdt) ----
        V8h = wpool.tile([128, L, E, ND, R], fp8)    # fp8(64*V)
        V8l = wpool.tile([128, L, E, ND, R], fp8)    # fp8(64*V - hi)
        Cr = wpool.tile([128, L, E, R], mmdt)        # C[l,e].T: [s128, r128]
        Ur = wpool.tile([128, L, E, ND, 128], mmdt)  # U[l,e].T kd-chunk: [r128, d128]

        def prep_V(l):
            vtmp = wtmp_p.tile([128, E, ND, R], f32, name=f"vtmp{l}", tag="wtmp")
            nc.gpsimd.dma_start(
                out=vtmp,
                in_=Vs_d[l].rearrange("e (kd p) r -> p e kd r", p=128))
            v64 = wtmp_p.tile([128, E, ND, R], f32, name=f"v64_{l}", tag="v64")
            nc.scalar.mul(v64, vtmp, WSCALE)
            nc.gpsimd.tensor_copy(V8h[:, l], v64)
            nc.vector.tensor_sub(V8l[:, l], v64, V8h[:, l])

        def prep_U(l):
            # U: [d, r] -> PE transpose to [r, d] chunks, batched 4-wide
            utmp = wtmp_p.tile([128, E, ND, R], f32, name=f"utmp{l}", tag="wtmp")
            nc.gpsimd.dma_start(
                out=utmp,
                in_=Us_d[l].rearrange("e (kd p) r -> p e kd r", p=128))
            for e in range(E):
                put = ptr.tile([128, 512], f32, name=f"put{l}_{e}", tag="tr")
                for kd in range(ND):
                    nc.tensor.transpose(
                        put[:, 128 * kd:128 * (kd + 1)], utmp[:, e, kd, :],
                        ident)
                nc.any.tensor_copy(
                    Ur[:, l, e].rearrange("p a b -> p (a b)"), put)

        def prep_C(l):
            # C: [r, s] -> [s, r], 4 experts batched into one psum tile
            ctmp = wtmp_p.tile([128, E, R], f32, name=f"ctmp{l}", tag="wtmp")
            nc.gpsimd.dma_start(out=ctmp, in_=Cs_d[l].rearrange("e r s -> r e s"))
            pct = ptr.tile([128, 512], f32, name=f"pct{l}", tag="tr")
            for e in range(E):
                nc.tensor.transpose(
                    pct[:, 128 * e:128 * (e + 1)], ctmp[:, e, :], ident)
            nc.any.tensor_copy(Cr[:, l].rearrange("p a b -> p (a b)"), pct)

        if has_bias:
            btmp = wtmp_p.tile([1, L * D], f32, name="btmp", tag="bias", bufs=1)
            nc.sync.dma_start(out=btmp,
                              in_=b_d[:].rearrange("l d -> (l d)")[None, :])
            bias4 = wpool.tile([1, L * D], mmdt)
            nc.scalar.mul(bias4, btmp, float(E))
            ones_t = wtmp_p.tile([1, 512], f32, name="ones_t", tag="ones1", bufs=1)
            nc.vector.memset(ones_t, 1.0)
            ones_r = wpool.tile([1, 512], mmdt)
            nc.vector.tensor_copy(ones_r, ones_t)

        # ---- x: natural load + PE transpose into [d, b] layout ----
        # Order: V(l=0) first so mm1 can start as soon as batch-group g=0 is
        # transposed; group-major transpose order so chunk c only needs the
        # first c+1 groups; x0r copied per group straight from PSUM.
        xlT = xpool.tile([128, ND, BL], f32)      # residual stream, fp32
        x0r = xpool.tile([128, ND, BL], mmdt)     # original x, matmul dtype
        with tc.tile_pool(name="xnat_p", bufs=1) as xnat_p:
            xnat = xnat_p.tile([128, NBT, D], f32)
            # first batch-group arrives in column chunks so the dc=0
            # transposes can start after 256KB instead of 1MB
            for dc in range(ND):
                for t in range(4):
                    nc.sync.dma_start(
                        out=xnat[:, t, 128 * dc:128 * (dc + 1)],
                        in_=x_d[128 * t:128 * (t + 1),
                                128 * dc:128 * (dc + 1)])
                if dc == 0:
                    prep_V(0)
                elif dc == 1:
                    prep_C(0)
            for t in range(4, NBT):
                nc.sync.dma_start(
                    out=xnat[:, t, :],
                    in_=x_d[128 * t:128 * (t + 1), :])
            for g in range(NBT // 4):
                for dc in range(ND):
                    pxt = ptr.tile([128, 512], f32, name=f"pxt{dc}_{g}",
                                   tag="tr")
                    for i in range(4):
                        nc.tensor.transpose(
                            pxt[:, 128 * i:128 * (i + 1)],
                            xnat[:, 4 * g + i, 128 * dc:128 * (dc + 1)],
                            ident)
                    nc.any.tensor_copy(
                        xlT[:, dc, 512 * g:512 * (g + 1)], pxt)
                    nc.any.tensor_copy(
                        x0r[:, dc, 512 * g:512 * (g + 1)], pxt)
                if g == 0:
                    prep_U(0)
                elif g == 1:
                    prep_V(1)
                    prep_C(1)
                elif g == 2:
                    prep_U(1)
                elif g == 3:
                    prep_V(2)
                    prep_C(2)
                    prep_U(2)

        # ---- main layer loop ----
        hz_p = ctx.enter_context(tc.tile_pool(name="hz_p", bufs=1))
        tmp_p = ctx.enter_context(tc.tile_pool(name="tmp_p", bufs=4))
        xlr_p = ctx.enter_context(tc.tile_pool(name="xlr_p", bufs=2))
        onat_p = ctx.enter_context(tc.tile_pool(name="onat_p", bufs=3))

        for l in range(L):
            for c in range(NBC):
                cols = slice(512 * c, 512 * (c + 1))
                x8h = xlr_p.tile([128, ND, 512], fp8,
                                 name=f"x8h{l}_{c}", tag="x8h")
                x8l = xlr_p.tile([128, ND, 512], fp8,
                                 name=f"x8l{l}_{c}", tag="x8l")
                nc.vector.tensor_copy(x8h, xlT[:, :, cols])
                nc.gpsimd.tensor_sub(x8l, xlT[:, :, cols], x8h)

                zr = []
                for e in range(E):
                    ph = ph_p.tile([128, 512], f32, name=f"ph{l}_{c}_{e}",
                                   tag="ph")
                    k = 0
                    for (W, X) in ((V8h, x8h), (V8l, x8h), (V8h, x8l)):
                        for kdp in range(2):
                            nc.tensor.matmul(
                                ph,
                                lhsT=W[:, l, e, 2 * kdp:2 * kdp + 2, :],
                                rhs=X[:, 2 * kdp:2 * kdp + 2, :],
                                start=(k == 0), stop=(k == 5),
                                perf_mode=DRmode)
                            k += 1
                    hr = hz_p.tile([128, 512], mmdt, name=f"h{l}_{c}_{e}",
                                   tag="h", bufs=6)
                    nc.scalar.activation(hr, ph, Tanh, scale=1.0 / WSCALE)

                    pz = pz_p.tile([128, 512], f32, name=f"pz{l}_{c}_{e}",
                                   tag="pz")
                    nc.tensor.matmul(pz, lhsT=Cr[:, l, e, :], rhs=hr,
                                     start=True, stop=True)
                    z = hz_p.tile([128, 512], mmdt, name=f"z{l}_{c}_{e}",
                                  tag="z", bufs=7 if has_bias else 8)
                    nc.scalar.activation(z, pz, Tanh)
                    zr.append(z)

                for dc in range(ND):
                    pu = pu_p.tile([128, 512], f32, name=f"pu{l}_{c}_{dc}",
                                   tag="pu")
                    for e in range(E):
                        nc.tensor.matmul(
                            pu, lhsT=Ur[:, l, e, dc, :], rhs=zr[e],
                            start=(e == 0),
                            stop=(e == E - 1 and not has_bias))
                    if has_bias:
                        nc.tensor.matmul(
                            pu,
                            lhsT=bias4[:, l * D + 128 * dc:l * D + 128 * (dc + 1)],
                            rhs=ones_r, start=False, stop=True)
                    tmp = tmp_p.tile([128, 512], f32, name=f"tmp{l}_{c}_{dc}",
                                     tag="tmp")
                    nc.vector.tensor_mul(
                        tmp, pu, x0r[:, dc, cols].bitcast(f32))
                    nc.vector.tensor_add(
                        xlT[:, dc, cols], xlT[:, dc, cols], tmp)

                if l == L - 1:
                    # store this chunk: transpose back to natural + DMA out
                    for t in range(4 * c, 4 * (c + 1)):
                        pot = ptr.tile([128, 512], f32, name=f"pot{t}",
                                       tag="tr")
                        for dc in range(ND):
                            nc.tensor.transpose(
                                pot[:, 128 * dc:128 * (dc + 1)],
                                xlT[:, dc, 128 * t:128 * (t + 1)], ident)
                        onat = onat_p.tile([128, D], f32, name=f"onat{t}",
                                           tag="onat")
                        nc.any.tensor_copy(onat, pot)
                        nc.sync.dma_start(
                            out=out_d[128 * t:128 * (t + 1), :], in_=onat)


    nc.finalize()
    return nc


def _get_prog(has_bias: bool, use_f32r: bool = True):
    key = (has_bias, use_f32r)
    if key not in _prog_cache:
        _prog_cache[key] = _build(has_bias, use_f32r)
    return _prog_cache[key]


def _run(inputs, trace=False, use_f32r=True):
    x = np.ascontiguousarray(np.asarray(inputs["x"], dtype=np.float32))
    Us = np.ascontiguousarray(np.asarray(inputs["Us"], dtype=np.float32))
    Cs = np.ascontiguousarray(np.asarray(inputs["Cs"], dtype=np.float32))
    Vs = np.ascontiguousarray(np.asarray(inputs["Vs"], dtype=np.float32))
    b = np.ascontiguousarray(np.asarray(inputs["b"], dtype=np.float32))
    assert x.shape == (B, D), x.shape
    has_bias = bool(np.any(b))
    nc = _get_prog(has_bias, use_f32r)
    shards = np.split(x, NCORES, axis=0)
    in_maps = []
    for i in range(NCORES):
        m = {"x": shards[i], "Us": Us, "Cs": Cs, "Vs": Vs}
        if has_bias:
            m["b"] = b
        in_maps.append(m)
    res = run_bass_kernel_spmd(nc, in_maps, core_ids=list(range(NCORES)),
                               trace=trace)
    out = np.concatenate([res.results[i]["out"] for i in range(NCORES)],
                         axis=0)
    return out, res


def kernel(**inputs) -> np.ndarray:
    out, _ = _run(inputs)
    return out



# revision 5
# speedup vs baseline: 1.0738x; 1.0220x over previous
"""Trainium2 Bass kernel for CrossNetGatingMixLayer.

Math (per layer i, with U,C,V per expert e; gate = softmax over a singleton
axis == 1.0 identically, so the gating einsum and G are dead code):

    xv = tanh(xl @ V[e])          (B,R)  per expert
    xc = tanh(xv @ C[e].T)        (B,R)
    xu = xc @ U[e].T              (B,D)
    xl = xl + x0 * (sum_e xu + E * bias)

Strategy: data-parallel over 8 NeuronCores (batch split 16384 -> 8 x 2048).
On-chip layout is transposed ([d, b]): all matmuls contract over d or r with
the contraction dim on SBUF partitions.  Matmuls run in float32r (4x faster
than fp32 on the PE; inputs rounded to 11 mantissa bits) while the residual
stream xl stays fp32.  x is transposed in/out via PE-transpose, batched in
groups of four 128x128 blocks per PSUM tile so eviction copies are wide.
"""
import numpy as np
from contextlib import ExitStack

import concourse.bass as bass
from concourse import bacc
import concourse.mybir as mybir
import concourse.tile as tile
from concourse.bass_utils import run_bass_kernel_spmd
from concourse.masks import make_identity

B, D, L, E, R = 16384, 512, 3, 4, 128
NCORES = 8
BL = B // NCORES            # 2048 rows per core
NBT = BL // 128             # 16 batch tiles of 128
NBC = BL // 512             # 4 batch chunks of 512 (matmul free dim)
ND = D // 128               # 4 d-chunks of 128
f32 = mybir.dt.float32
f32r = mybir.dt.float32r
bf16 = mybir.dt.bfloat16
fp8 = mybir.dt.float8e4
DRmode = mybir.MatmulPerfMode.DoubleRow
WSCALE = 64.0
Tanh = mybir.ActivationFunctionType.Tanh

_prog_cache = {}


def _build(has_bias: bool, use_f32r: bool):
    mmdt = f32r if use_f32r else f32
    nc = bacc.Bacc("TRN2")
    x_d = nc.declare_dram_parameter("x", [BL, D], f32, isOutput=False)
    Vs_d = nc.declare_dram_parameter("Vs", [L, E, D, R], f32, isOutput=False)
    Cs_d = nc.declare_dram_parameter("Cs", [L, E, R, R], f32, isOutput=False)
    Us_d = nc.declare_dram_parameter("Us", [L, E, D, R], f32, isOutput=False)
    if has_bias:
        b_d = nc.declare_dram_parameter("b", [L, D], f32, isOutput=False)
    out_d = nc.declare_dram_parameter("out", [BL, D], f32, isOutput=True)

    with tile.TileContext(nc) as tc, ExitStack() as ctx:
        const = ctx.enter_context(tc.tile_pool(name="const", bufs=1))
        wpool = ctx.enter_context(tc.tile_pool(name="wpool", bufs=1))
        xpool = ctx.enter_context(tc.tile_pool(name="xpool", bufs=1))
        wtmp_p = ctx.enter_context(tc.tile_pool(name="wtmp_p", bufs=2))
        ptr = ctx.enter_context(tc.tile_pool(name="ptr", bufs=2, space="PSUM"))
        ph_p = ctx.enter_context(tc.tile_pool(name="ph_p", bufs=3, space="PSUM"))
        pz_p = ctx.enter_context(tc.tile_pool(name="pz_p", bufs=1, space="PSUM"))
        pu_p = ctx.enter_context(tc.tile_pool(name="pu_p", bufs=2, space="PSUM"))

        ident = const.tile([128, 128], f32)
        make_identity(nc, ident)

        # ---- persistent weight tiles (mmdt) ----
        V8h = wpool.tile([128, L, E, ND, R], fp8)    # fp8(64*V)
        V8l = wpool.tile([128, L, E, ND, R], fp8)    # fp8(64*V - hi)
        Cr = wpool.tile([128, L, E, R], mmdt)        # C[l,e].T: [s128, r128]
        Ur = wpool.tile([128, L, E, ND, 128], mmdt)  # U[l,e].T kd-chunk: [r128, d128]

        def prep_V(l):
            vtmp = wtmp_p.tile([128, E, ND, R], f32, name=f"vtmp{l}", tag="wtmp")
            nc.gpsimd.dma_start(
                out=vtmp,
                in_=Vs_d[l].rearrange("e (kd p) r -> p e kd r", p=128))
            v64 = wtmp_p.tile([128, E, ND, R], f32, name=f"v64_{l}", tag="v64", bufs=1)
            nc.scalar.mul(v64, vtmp, WSCALE)
            nc.gpsimd.tensor_copy(V8h[:, l], v64)
            nc.vector.tensor_sub(V8l[:, l], v64, V8h[:, l])

        def prep_U(l):
            # U: [d, r] -> PE transpose to [r, d] chunks, batched 4-wide
            utmp = wtmp_p.tile([128, E, ND, R], f32, name=f"utmp{l}", tag="wtmp")
            nc.gpsimd.dma_start(
                out=utmp,
                in_=Us_d[l].rearrange("e (kd p) r -> p e kd r", p=128))
            for e in range(E):
                put = ptr.tile([128, 512], f32, name=f"put{l}_{e}", tag="tr")
                for kd in range(ND):
                    nc.tensor.transpose(
                        put[:, 128 * kd:128 * (kd + 1)], utmp[:, e, kd, :],
                        ident)
                nc.any.tensor_copy(
                    Ur[:, l, e].rearrange("p a b -> p (a b)"), put)

        def prep_C(l):
            # C: [r, s] -> [s, r], 4 experts batched into one psum tile
            ctmp = wtmp_p.tile([128, E, R], f32, name=f"ctmp{l}", tag="wtmp")
            nc.gpsimd.dma_start(out=ctmp, in_=Cs_d[l].rearrange("e r s -> r e s"))
            pct = ptr.tile([128, 512], f32, name=f"pct{l}", tag="tr")
            for e in range(E):
                nc.tensor.transpose(
                    pct[:, 128 * e:128 * (e + 1)], ctmp[:, e, :], ident)
            nc.any.tensor_copy(Cr[:, l].rearrange("p a b -> p (a b)"), pct)

        if has_bias:
            btmp = wtmp_p.tile([1, L * D], f32, name="btmp", tag="bias", bufs=1)
            nc.sync.dma_start(out=btmp,
                              in_=b_d[:].rearrange("l d -> (l d)")[None, :])
            bias4 = wpool.tile([1, L * D], mmdt)
            nc.scalar.mul(bias4, btmp, float(E))
            ones_t = wtmp_p.tile([1, 512], f32, name="ones_t", tag="ones1", bufs=1)
            nc.vector.memset(ones_t, 1.0)
            ones_r = wpool.tile([1, 512], mmdt)
            nc.vector.tensor_copy(ones_r, ones_t)

        hz_p = ctx.enter_context(tc.tile_pool(name="hz_p", bufs=1))
        tmp_p = ctx.enter_context(tc.tile_pool(name="tmp_p", bufs=4))
        xlr_p = ctx.enter_context(tc.tile_pool(name="xlr_p", bufs=2))
        onat_p = ctx.enter_context(tc.tile_pool(name="onat_p", bufs=3))
        x8tiles = {}

        def emit_split(l, c):
            cols = slice(512 * c, 512 * (c + 1))
            x8h = xlr_p.tile([128, ND, 512], fp8,
                             name=f"x8h{l}_{c}", tag="x8h", bufs=2)
            x8l = xlr_p.tile([128, ND, 512], fp8,
                             name=f"x8l{l}_{c}", tag="x8l", bufs=2)
            nc.vector.tensor_copy(x8h, xlT[:, :, cols])
            eng = nc.vector if l == 0 else nc.gpsimd
            eng.tensor_sub(x8l, xlT[:, :, cols], x8h)
            x8tiles[(l, c)] = (x8h, x8l)

        # ---- x: natural load + PE transpose into [d, b] layout ----
        # Order: V(l=0) first so mm1 can start as soon as batch-group g=0 is
        # transposed; group-major transpose order so chunk c only needs the
        # first c+1 groups; x0r copied per group straight from PSUM.
        xlT = xpool.tile([128, ND, BL], f32)      # residual stream, fp32
        x0r = xpool.tile([128, ND, BL], bf16)     # original x (mult operand)
        bf16_unused = None
        with tc.tile_pool(name="xnat_p", bufs=1) as xnat_p:
            xnat = xnat_p.tile([128, NBT, D], f32)
            # first batch-group arrives in column chunks so the dc=0
            # transposes can start after 256KB instead of 1MB
            for dc in range(ND):
                for t in range(4):
                    nc.sync.dma_start(
                        out=xnat[:, t, 128 * dc:128 * (dc + 1)],
                        in_=x_d[128 * t:128 * (t + 1),
                                128 * dc:128 * (dc + 1)])
                if dc == 0:
                    prep_V(0)
                elif dc == 1:
                    prep_C(0)
            for t in range(4, NBT):
                nc.sync.dma_start(
                    out=xnat[:, t, :],
                    in_=x_d[128 * t:128 * (t + 1), :])
            for g in range(NBT // 4):
                for dc in range(ND):
                    pxt = ptr.tile([128, 512], f32, name=f"pxt{dc}_{g}",
                                   tag="tr")
                    for i in range(4):
                        nc.tensor.transpose(
                            pxt[:, 128 * i:128 * (i + 1)],
                            xnat[:, 4 * g + i, 128 * dc:128 * (dc + 1)],
                            ident)
                    nc.any.tensor_copy(
                        xlT[:, dc, 512 * g:512 * (g + 1)], pxt)
                    nc.any.tensor_copy(
                        x0r[:, dc, 512 * g:512 * (g + 1)], pxt)
                emit_split(0, g)
                if g == 0:
                    prep_U(0)
                elif g == 1:
                    prep_V(1)
                    prep_C(1)
                elif g == 2:
                    prep_U(1)
                elif g == 3:
                    prep_V(2)
                    prep_C(2)
                    prep_U(2)

        # ---- main layer loop ----
        for l in range(L):
            for c in range(NBC):
                cols = slice(512 * c, 512 * (c + 1))
                x8h, x8l = x8tiles.pop((l, c))

                zr = []
                for e in range(E):
                    ph = ph_p.tile([128, 512], f32, name=f"ph{l}_{c}_{e}",
                                   tag="ph")
                    k = 0
                    for (W, X) in ((V8h, x8h), (V8l, x8h), (V8h, x8l)):
                        for kdp in range(2):
                            nc.tensor.matmul(
                                ph,
                                lhsT=W[:, l, e, 2 * kdp:2 * kdp + 2, :],
                                rhs=X[:, 2 * kdp:2 * kdp + 2, :],
                                start=(k == 0), stop=(k == 5),
                                perf_mode=DRmode)
                            k += 1
                    hr = hz_p.tile([128, 512], mmdt, name=f"h{l}_{c}_{e}",
                                   tag="h", bufs=5)
                    nc.scalar.activation(hr, ph, Tanh, scale=1.0 / WSCALE)

                    pz = pz_p.tile([128, 512], f32, name=f"pz{l}_{c}_{e}",
                                   tag="pz")
                    nc.tensor.matmul(pz, lhsT=Cr[:, l, e, :], rhs=hr,
                                     start=True, stop=True)
                    z = hz_p.tile([128, 512], mmdt, name=f"z{l}_{c}_{e}",
                                  tag="z", bufs=6 if has_bias else 7)
                    nc.scalar.activation(z, pz, Tanh)
                    zr.append(z)

                for dc in range(ND):
                    pu = pu_p.tile([128, 512], f32, name=f"pu{l}_{c}_{dc}",
                                   tag="pu")
                    for e in range(E):
                        nc.tensor.matmul(
                            pu, lhsT=Ur[:, l, e, dc, :], rhs=zr[e],
                            start=(e == 0),
                            stop=(e == E - 1 and not has_bias))
                    if has_bias:
                        nc.tensor.matmul(
                            pu,
                            lhsT=bias4[:, l * D + 128 * dc:l * D + 128 * (dc + 1)],
                            rhs=ones_r, start=False, stop=True)
                    tmp = tmp_p.tile([128, 512], f32, name=f"tmp{l}_{c}_{dc}",
                                     tag="tmp")
                    nc.vector.tensor_mul(
                        tmp, pu, x0r[:, dc, cols])
                    nc.vector.tensor_add(
                        xlT[:, dc, cols], xlT[:, dc, cols], tmp)
                if l + 1 < L:
                    emit_split(l + 1, c)

                if l == L - 1:
                    # store this chunk: transpose back to natural + DMA out
                    for t in range(4 * c, 4 * (c + 1)):
                        pot = ptr.tile([128, 512], f32, name=f"pot{t}",
                                       tag="tr")
                        for dc in range(ND):
                            nc.tensor.transpose(
                                pot[:, 128 * dc:128 * (dc + 1)],
                                xlT[:, dc, 128 * t:128 * (t + 1)], ident)
                        onat = onat_p.tile([128, D], f32, name=f"onat{t}",
                                           tag="onat")
                        nc.any.tensor_copy(onat, pot)
                        nc.sync.dma_start(
                            out=out_d[128 * t:128 * (t + 1), :], in_=onat)


    nc.finalize()
    return nc


def _get_prog(has_bias: bool, use_f32r: bool = True):
    key = (has_bias, use_f32r)
    if key not in _prog_cache:
        _prog_cache[key] = _build(has_bias, use_f32r)
    return _prog_cache[key]


def _run(inputs, trace=False, use_f32r=True):
    x = np.ascontiguousarray(np.asarray(inputs["x"], dtype=np.float32))
    Us = np.ascontiguousarray(np.asarray(inputs["Us"], dtype=np.float32))
    Cs = np.ascontiguousarray(np.asarray(inputs["Cs"], dtype=np.float32))
    Vs = np.ascontiguousarray(np.asarray(inputs["Vs"], dtype=np.float32))
    b = np.ascontiguousarray(np.asarray(inputs["b"], dtype=np.float32))
    assert x.shape == (B, D), x.shape
    has_bias = bool(np.any(b))
    nc = _get_prog(has_bias, use_f32r)
    shards = np.split(x, NCORES, axis=0)
    in_maps = []
    for i in range(NCORES):
        m = {"x": shards[i], "Us": Us, "Cs": Cs, "Vs": Vs}
        if has_bias:
            m["b"] = b
        in_maps.append(m)
    res = run_bass_kernel_spmd(nc, in_maps, core_ids=list(range(NCORES)),
                               trace=trace)
    out = np.concatenate([res.results[i]["out"] for i in range(NCORES)],
                         axis=0)
    return out, res


def kernel(**inputs) -> np.ndarray:
    out, _ = _run(inputs)
    return out



# revision 6
# speedup vs baseline: 1.1096x; 1.0334x over previous
"""Trainium2 Bass kernel for CrossNetGatingMixLayer.

Math (per layer i, with U,C,V per expert e; gate = softmax over a singleton
axis == 1.0 identically, so the gating einsum and G are dead code):

    xv = tanh(xl @ V[e])          (B,R)  per expert
    xc = tanh(xv @ C[e].T)        (B,R)
    xu = xc @ U[e].T              (B,D)
    xl = xl + x0 * (sum_e xu + E * bias)

Strategy: data-parallel over 8 NeuronCores (batch split 16384 -> 8 x 2048).
On-chip layout is transposed ([d, b]): all matmuls contract over d or r with
the contraction dim on SBUF partitions.  Matmuls run in float32r (4x faster
than fp32 on the PE; inputs rounded to 11 mantissa bits) while the residual
stream xl stays fp32.  x is transposed in/out via PE-transpose, batched in
groups of four 128x128 blocks per PSUM tile so eviction copies are wide.
"""
import numpy as np
from contextlib import ExitStack

import concourse.bass as bass
from concourse import bacc
import concourse.mybir as mybir
import concourse.tile as tile
from concourse.bass_utils import run_bass_kernel_spmd
from concourse.masks import make_identity

B, D, L, E, R = 16384, 512, 3, 4, 128
NCORES = 8
BL = B // NCORES            # 2048 rows per core
NBT = BL // 128             # 16 batch tiles of 128
NBC = BL // 512             # 4 batch chunks of 512 (matmul free dim)
ND = D // 128               # 4 d-chunks of 128
f32 = mybir.dt.float32
f32r = mybir.dt.float32r
bf16 = mybir.dt.bfloat16
fp8 = mybir.dt.float8e4
DRmode = mybir.MatmulPerfMode.DoubleRow
WSCALE = 64.0
Tanh = mybir.ActivationFunctionType.Tanh

_prog_cache = {}


def _build(has_bias: bool, use_f32r: bool):
    mmdt = f32r if use_f32r else f32
    nc = bacc.Bacc("TRN2")
    x_d = nc.declare_dram_parameter("x", [BL, D], f32, isOutput=False)
    Vs_d = nc.declare_dram_parameter("Vs", [L, E, D, R], f32, isOutput=False)
    Cs_d = nc.declare_dram_parameter("Cs", [L, E, R, R], f32, isOutput=False)
    Us_d = nc.declare_dram_parameter("Us", [L, E, D, R], f32, isOutput=False)
    if has_bias:
        b_d = nc.declare_dram_parameter("b", [L, D], f32, isOutput=False)
    out_d = nc.declare_dram_parameter("out", [BL, D], f32, isOutput=True)

    with tile.TileContext(nc) as tc, ExitStack() as ctx:
        const = ctx.enter_context(tc.tile_pool(name="const", bufs=1))
        wpool = ctx.enter_context(tc.tile_pool(name="wpool", bufs=1))
        xpool = ctx.enter_context(tc.tile_pool(name="xpool", bufs=1))
        wtmp_p = ctx.enter_context(tc.tile_pool(name="wtmp_p", bufs=2))
        ptr = ctx.enter_context(tc.tile_pool(name="ptr", bufs=2, space="PSUM"))
        ph_p = ctx.enter_context(tc.tile_pool(name="ph_p", bufs=3, space="PSUM"))
        pz_p = ctx.enter_context(tc.tile_pool(name="pz_p", bufs=1, space="PSUM"))
        pu_p = ctx.enter_context(tc.tile_pool(name="pu_p", bufs=2, space="PSUM"))

        ident = const.tile([128, 128], f32)
        make_identity(nc, ident)
        identb = const.tile([128, 128], bf16)
        make_identity(nc, identb)

        # ---- persistent weight tiles (mmdt) ----
        V8h = wpool.tile([128, L, E, ND, R], fp8)    # fp8(64*V)
        V8l = wpool.tile([128, L, E, ND, R], fp8)    # fp8(64*V - hi)
        Cr = wpool.tile([128, L, E, R], mmdt)        # C[l,e].T: [s128, r128]
        Ur = wpool.tile([128, L, E, ND, 128], mmdt)  # U[l,e].T kd-chunk: [r128, d128]

        def prep_V(l):
            vtmp = wtmp_p.tile([128, E, ND, R], f32, name=f"vtmp{l}", tag="wtmp")
            nc.gpsimd.dma_start(
                out=vtmp,
                in_=Vs_d[l].rearrange("e (kd p) r -> p e kd r", p=128))
            v64 = wtmp_p.tile([128, E, ND, R], f32, name=f"v64_{l}", tag="v64", bufs=1)
            nc.scalar.mul(v64, vtmp, WSCALE)
            nc.gpsimd.tensor_copy(V8h[:, l], v64)
            nc.vector.tensor_sub(V8l[:, l], v64, V8h[:, l])

        def prep_U(l):
            # U: [d, r] -> PE transpose to [r, d] chunks, batched 4-wide
            utmp = wtmp_p.tile([128, E, ND, R], f32, name=f"utmp{l}", tag="wtmp")
            nc.gpsimd.dma_start(
                out=utmp,
                in_=Us_d[l].rearrange("e (kd p) r -> p e kd r", p=128))
            for e in range(E):
                put = ptr.tile([128, 512], f32, name=f"put{l}_{e}", tag="tr")
                for kd in range(ND):
                    nc.tensor.transpose(
                        put[:, 128 * kd:128 * (kd + 1)], utmp[:, e, kd, :],
                        ident)
                nc.any.tensor_copy(
                    Ur[:, l, e].rearrange("p a b -> p (a b)"), put)

        def prep_C(l):
            # C: [r, s] -> [s, r], 4 experts batched into one psum tile
            ctmp = wtmp_p.tile([128, E, R], f32, name=f"ctmp{l}", tag="wtmp")
            nc.gpsimd.dma_start(out=ctmp, in_=Cs_d[l].rearrange("e r s -> r e s"))
            pct = ptr.tile([128, 512], f32, name=f"pct{l}", tag="tr")
            for e in range(E):
                nc.tensor.transpose(
                    pct[:, 128 * e:128 * (e + 1)], ctmp[:, e, :], ident)
            nc.any.tensor_copy(Cr[:, l].rearrange("p a b -> p (a b)"), pct)

        if has_bias:
            btmp = wtmp_p.tile([1, L * D], f32, name="btmp", tag="bias", bufs=1)
            nc.sync.dma_start(out=btmp,
                              in_=b_d[:].rearrange("l d -> (l d)")[None, :])
            bias4 = wpool.tile([1, L * D], mmdt)
            nc.scalar.mul(bias4, btmp, float(E))
            ones_t = wtmp_p.tile([1, 512], f32, name="ones_t", tag="ones1", bufs=1)
            nc.vector.memset(ones_t, 1.0)
            ones_r = wpool.tile([1, 512], mmdt)
            nc.vector.tensor_copy(ones_r, ones_t)

        hz_p = ctx.enter_context(tc.tile_pool(name="hz_p", bufs=1))
        tmp_p = ctx.enter_context(tc.tile_pool(name="tmp_p", bufs=4))
        xlr_p = ctx.enter_context(tc.tile_pool(name="xlr_p", bufs=2))
        onat_p = ctx.enter_context(tc.tile_pool(name="onat_p", bufs=3))
        x8tiles = {}

        def emit_split(l, c):
            cols = slice(512 * c, 512 * (c + 1))
            x8h = xlr_p.tile([128, ND, 512], fp8,
                             name=f"x8h{l}_{c}", tag="x8h", bufs=3)
            x8l = xlr_p.tile([128, ND, 512], fp8,
                             name=f"x8l{l}_{c}", tag="x8l", bufs=3)
            nc.vector.tensor_copy(x8h, xlT[:, :, cols])
            eng = nc.vector if l == 0 else nc.gpsimd
            eng.tensor_sub(x8l, xlT[:, :, cols], x8h)
            x8tiles[(l, c)] = (x8h, x8l)

        # ---- x: natural load + PE transpose into [d, b] layout ----
        # Order: V(l=0) first so mm1 can start as soon as batch-group g=0 is
        # transposed; group-major transpose order so chunk c only needs the
        # first c+1 groups; x0r copied per group straight from PSUM.
        xlT = xpool.tile([128, ND, BL], bf16)     # residual stream, bf16
        x0r = xpool.tile([128, ND, BL], bf16)     # original x (mult operand)
        bf16_unused = None
        with tc.tile_pool(name="xnat_p", bufs=1) as xnat_p:
            xnat = xnat_p.tile([128, NBT, D], f32)
            # first batch-group arrives in column chunks so the dc=0
            # transposes can start after 256KB instead of 1MB
            for dc in range(ND):
                for t in range(4):
                    nc.sync.dma_start(
                        out=xnat[:, t, 128 * dc:128 * (dc + 1)],
                        in_=x_d[128 * t:128 * (t + 1),
                                128 * dc:128 * (dc + 1)])
                if dc == 0:
                    prep_V(0)
                elif dc == 1:
                    prep_C(0)
            for t in range(4, NBT):
                nc.sync.dma_start(
                    out=xnat[:, t, :],
                    in_=x_d[128 * t:128 * (t + 1), :])
            for g in range(NBT // 4):
                for dc in range(ND):
                    pxt = ptr.tile([128, 512], f32, name=f"pxt{dc}_{g}",
                                   tag="tr")
                    for i in range(4):
                        nc.tensor.transpose(
                            pxt[:, 128 * i:128 * (i + 1)],
                            xnat[:, 4 * g + i, 128 * dc:128 * (dc + 1)],
                            ident)
                    nc.any.tensor_copy(
                        xlT[:, dc, 512 * g:512 * (g + 1)], pxt)
                    nc.any.tensor_copy(
                        x0r[:, dc, 512 * g:512 * (g + 1)], pxt)
                emit_split(0, g)
                if g == 0:
                    prep_U(0)
                elif g == 1:
                    prep_V(1)
                    prep_C(1)
                elif g == 2:
                    prep_U(1)
                elif g == 3:
                    prep_V(2)
                    prep_C(2)
                    prep_U(2)

        # ---- main layer loop ----
        for l in range(L):
            for c in range(NBC):
                cols = slice(512 * c, 512 * (c + 1))
                x8h, x8l = x8tiles.pop((l, c))

                zr = []
                for e in range(E):
                    ph = ph_p.tile([128, 512], f32, name=f"ph{l}_{c}_{e}",
                                   tag="ph")
                    k = 0
                    for (W, X) in ((V8h, x8h), (V8l, x8h), (V8h, x8l)):
                        for kdp in range(2):
                            nc.tensor.matmul(
                                ph,
                                lhsT=W[:, l, e, 2 * kdp:2 * kdp + 2, :],
                                rhs=X[:, 2 * kdp:2 * kdp + 2, :],
                                start=(k == 0), stop=(k == 5),
                                perf_mode=DRmode)
                            k += 1
                    hr = hz_p.tile([128, 512], mmdt, name=f"h{l}_{c}_{e}",
                                   tag="h", bufs=6)
                    nc.scalar.activation(hr, ph, Tanh, scale=1.0 / WSCALE)

                    pz = pz_p.tile([128, 512], f32, name=f"pz{l}_{c}_{e}",
                                   tag="pz")
                    nc.tensor.matmul(pz, lhsT=Cr[:, l, e, :], rhs=hr,
                                     start=True, stop=True)
                    z = hz_p.tile([128, 512], mmdt, name=f"z{l}_{c}_{e}",
                                  tag="z", bufs=7 if has_bias else 8)
                    nc.scalar.activation(z, pz, Tanh)
                    zr.append(z)

                for dc in range(ND):
                    pu = pu_p.tile([128, 512], f32, name=f"pu{l}_{c}_{dc}",
                                   tag="pu")
                    for e in range(E):
                        nc.tensor.matmul(
                            pu, lhsT=Ur[:, l, e, dc, :], rhs=zr[e],
                            start=(e == 0),
                            stop=(e == E - 1 and not has_bias))
                    if has_bias:
                        nc.tensor.matmul(
                            pu,
                            lhsT=bias4[:, l * D + 128 * dc:l * D + 128 * (dc + 1)],
                            rhs=ones_r, start=False, stop=True)
                    tmp = tmp_p.tile([128, 512], bf16, name=f"tmp{l}_{c}_{dc}",
                                     tag="tmp")
                    nc.vector.tensor_mul(
                        tmp, pu, x0r[:, dc, cols])
                    nc.vector.tensor_add(
                        xlT[:, dc, cols], xlT[:, dc, cols], tmp)
                if l + 1 < L:
                    emit_split(l + 1, c)

                if l == L - 1:
                    # store this chunk: transpose back to natural + DMA out
                    for t in range(4 * c, 4 * (c + 1)):
                        pot = ptr.tile([128, 512], bf16, name=f"pot{t}",
                                       tag="tr")
                        for dc in range(ND):
                            nc.tensor.transpose(
                                pot[:, 128 * dc:128 * (dc + 1)],
                                xlT[:, dc, 128 * t:128 * (t + 1)], identb)
                        onat = onat_p.tile([128, D], f32, name=f"onat{t}",
                                           tag="onat")
                        nc.any.tensor_copy(onat, pot)
                        nc.sync.dma_start(
                            out=out_d[128 * t:128 * (t + 1), :], in_=onat)


    nc.finalize()
    return nc


def _get_prog(has_bias: bool, use_f32r: bool = True):
    key = (has_bias, use_f32r)
    if key not in _prog_cache:
        _prog_cache[key] = _build(has_bias, use_f32r)
    return _prog_cache[key]


def _run(inputs, trace=False, use_f32r=True):
    x = np.ascontiguousarray(np.asarray(inputs["x"], dtype=np.float32))
    Us = np.ascontiguousarray(np.asarray(inputs["Us"], dtype=np.float32))
    Cs = np.ascontiguousarray(np.asarray(inputs["Cs"], dtype=np.float32))
    Vs = np.ascontiguousarray(np.asarray(inputs["Vs"], dtype=np.float32))
    b = np.ascontiguousarray(np.asarray(inputs["b"], dtype=np.float32))
    assert x.shape == (B, D), x.shape
    has_bias = bool(np.any(b))
    nc = _get_prog(has_bias, use_f32r)
    shards = np.split(x, NCORES, axis=0)
    in_maps = []
    for i in range(NCORES):
        m = {"x": shards[i], "Us": Us, "Cs": Cs, "Vs": Vs}
        if has_bias:
            m["b"] = b
        in_maps.append(m)
    res = run_bass_kernel_spmd(nc, in_maps, core_ids=list(range(NCORES)),
                               trace=trace)
    out = np.concatenate([res.results[i]["out"] for i in range(NCORES)],
                         axis=0)
    return out, res


def kernel(**inputs) -> np.ndarray:
    out, _ = _run(inputs)
    return out



# revision 7
# speedup vs baseline: 1.1141x; 1.0041x over previous
"""Trainium2 Bass kernel for CrossNetGatingMixLayer.

Math (per layer i, with U,C,V per expert e; gate = softmax over a singleton
axis == 1.0 identically, so the gating einsum and G are dead code):

    xv = tanh(xl @ V[e])          (B,R)  per expert
    xc = tanh(xv @ C[e].T)        (B,R)
    xu = xc @ U[e].T              (B,D)
    xl = xl + x0 * (sum_e xu + E * bias)

Strategy: data-parallel over 8 NeuronCores (batch split 16384 -> 8 x 2048).
On-chip layout is transposed ([d, b]): all matmuls contract over d or r with
the contraction dim on SBUF partitions.  Matmuls run in float32r (4x faster
than fp32 on the PE; inputs rounded to 11 mantissa bits) while the residual
stream xl stays fp32.  x is transposed in/out via PE-transpose, batched in
groups of four 128x128 blocks per PSUM tile so eviction copies are wide.
"""
import numpy as np
from contextlib import ExitStack

import concourse.bass as bass
from concourse import bacc
import concourse.mybir as mybir
import concourse.tile as tile
from concourse.bass_utils import run_bass_kernel_spmd
from concourse.masks import make_identity

B, D, L, E, R = 16384, 512, 3, 4, 128
NCORES = 8
BL = B // NCORES            # 2048 rows per core
NBT = BL // 128             # 16 batch tiles of 128
NBC = BL // 512             # 4 batch chunks of 512 (matmul free dim)
ND = D // 128               # 4 d-chunks of 128
f32 = mybir.dt.float32
f32r = mybir.dt.float32r
bf16 = mybir.dt.bfloat16
fp8 = mybir.dt.float8e4
DRmode = mybir.MatmulPerfMode.DoubleRow
WSCALE = 64.0
Tanh = mybir.ActivationFunctionType.Tanh

_prog_cache = {}


def _build(has_bias: bool, use_f32r: bool):
    mmdt = f32r if use_f32r else f32
    nc = bacc.Bacc("TRN2")
    x_d = nc.declare_dram_parameter("x", [BL, D], f32, isOutput=False)
    Vs_d = nc.declare_dram_parameter("Vs", [L, E, D, R], f32, isOutput=False)
    Cs_d = nc.declare_dram_parameter("Cs", [L, E, R, R], f32, isOutput=False)
    Us_d = nc.declare_dram_parameter("Us", [L, E, D, R], f32, isOutput=False)
    if has_bias:
        b_d = nc.declare_dram_parameter("b", [L, D], f32, isOutput=False)
    out_d = nc.declare_dram_parameter("out", [BL, D], f32, isOutput=True)

    with tile.TileContext(nc) as tc, ExitStack() as ctx:
        const = ctx.enter_context(tc.tile_pool(name="const", bufs=1))
        wpool = ctx.enter_context(tc.tile_pool(name="wpool", bufs=1))
        xpool = ctx.enter_context(tc.tile_pool(name="xpool", bufs=1))
        wtmp_p = ctx.enter_context(tc.tile_pool(name="wtmp_p", bufs=2))
        ptr = ctx.enter_context(tc.tile_pool(name="ptr", bufs=2, space="PSUM"))
        ph_p = ctx.enter_context(tc.tile_pool(name="ph_p", bufs=3, space="PSUM"))
        pz_p = ctx.enter_context(tc.tile_pool(name="pz_p", bufs=1, space="PSUM"))
        pu_p = ctx.enter_context(tc.tile_pool(name="pu_p", bufs=2, space="PSUM"))

        ident = const.tile([128, 128], f32)
        make_identity(nc, ident)
        identb = const.tile([128, 128], bf16)
        make_identity(nc, identb)

        # ---- persistent weight tiles (mmdt) ----
        V8h = wpool.tile([128, L, E, ND, R], fp8)    # fp8(64*V)
        V8l = wpool.tile([128, L, E, ND, R], fp8)    # fp8(64*V - hi)
        Cr = wpool.tile([128, L, E, R], mmdt)        # C[l,e].T: [s128, r128]
        Ur = wpool.tile([128, L, E, ND, 128], mmdt)  # U[l,e].T kd-chunk: [r128, d128]

        def prep_V(l):
            vtmp = wtmp_p.tile([128, E, ND, R], f32, name=f"vtmp{l}", tag="wtmp")
            nc.gpsimd.dma_start(
                out=vtmp,
                in_=Vs_d[l].rearrange("e (kd p) r -> p e kd r", p=128))
            v64 = wtmp_p.tile([128, E, ND, R], f32, name=f"v64_{l}", tag="v64", bufs=1)
            nc.scalar.mul(v64, vtmp, WSCALE)
            nc.gpsimd.tensor_copy(V8h[:, l], v64)
            nc.vector.tensor_sub(V8l[:, l], v64, V8h[:, l])

        v0holder = []

        def prep_V0(experts):
            if not v0holder:
                v0holder.append(wtmp_p.tile(
                    [128, E, ND, R], f32, name="v0tmp", tag="wtmp"))
            v0tmp = v0holder[0]
            for e in experts:
                nc.gpsimd.dma_start(
                    out=v0tmp[:, e],
                    in_=Vs_d[0, e].rearrange("(kd p) r -> p kd r", p=128))
                v64e = wtmp_p.tile([128, ND, R], f32, name=f"v064_{e}",
                                   tag="v64e", bufs=2)
                nc.scalar.mul(v64e, v0tmp[:, e], WSCALE)
                nc.vector.tensor_copy(V8h[:, 0, e], v64e)
                nc.vector.tensor_sub(V8l[:, 0, e], v64e, V8h[:, 0, e])

        def prep_U(l):
            # U: [d, r] -> PE transpose to [r, d] chunks, batched 4-wide
            utmp = wtmp_p.tile([128, E, ND, R], f32, name=f"utmp{l}", tag="wtmp")
            nc.gpsimd.dma_start(
                out=utmp,
                in_=Us_d[l].rearrange("e (kd p) r -> p e kd r", p=128))
            for e in range(E):
                put = ptr.tile([128, 512], f32, name=f"put{l}_{e}", tag="tr")
                for kd in range(ND):
                    nc.tensor.transpose(
                        put[:, 128 * kd:128 * (kd + 1)], utmp[:, e, kd, :],
                        ident)
                nc.any.tensor_copy(
                    Ur[:, l, e].rearrange("p a b -> p (a b)"), put)

        def prep_C(l):
            # C: [r, s] -> [s, r], 4 experts batched into one psum tile
            ctmp = wtmp_p.tile([128, E, R], f32, name=f"ctmp{l}", tag="wtmp")
            nc.gpsimd.dma_start(out=ctmp, in_=Cs_d[l].rearrange("e r s -> r e s"))
            pct = ptr.tile([128, 512], f32, name=f"pct{l}", tag="tr")
            for e in range(E):
                nc.tensor.transpose(
                    pct[:, 128 * e:128 * (e + 1)], ctmp[:, e, :], ident)
            nc.any.tensor_copy(Cr[:, l].rearrange("p a b -> p (a b)"), pct)

        if has_bias:
            btmp = wtmp_p.tile([1, L * D], f32, name="btmp", tag="bias", bufs=1)
            nc.sync.dma_start(out=btmp,
                              in_=b_d[:].rearrange("l d -> (l d)")[None, :])
            bias4 = wpool.tile([1, L * D], mmdt)
            nc.scalar.mul(bias4, btmp, float(E))
            ones_t = wtmp_p.tile([1, 512], f32, name="ones_t", tag="ones1", bufs=1)
            nc.vector.memset(ones_t, 1.0)
            ones_r = wpool.tile([1, 512], mmdt)
            nc.vector.tensor_copy(ones_r, ones_t)

        hz_p = ctx.enter_context(tc.tile_pool(name="hz_p", bufs=1))
        tmp_p = ctx.enter_context(tc.tile_pool(name="tmp_p", bufs=4))
        xlr_p = ctx.enter_context(tc.tile_pool(name="xlr_p", bufs=2))
        onat_p = ctx.enter_context(tc.tile_pool(name="onat_p", bufs=3))
        x8tiles = {}

        def emit_split(l, c):
            cols = slice(512 * c, 512 * (c + 1))
            x8h = xlr_p.tile([128, ND, 512], fp8,
                             name=f"x8h{l}_{c}", tag="x8h", bufs=3)
            x8l = xlr_p.tile([128, ND, 512], fp8,
                             name=f"x8l{l}_{c}", tag="x8l", bufs=3)
            nc.vector.tensor_copy(x8h, xlT[:, :, cols])
            eng = nc.vector if l == 0 else nc.gpsimd
            eng.tensor_sub(x8l, xlT[:, :, cols], x8h)
            x8tiles[(l, c)] = (x8h, x8l)

        # ---- x: natural load + PE transpose into [d, b] layout ----
        # Order: V(l=0) first so mm1 can start as soon as batch-group g=0 is
        # transposed; group-major transpose order so chunk c only needs the
        # first c+1 groups; x0r copied per group straight from PSUM.
        xlT = xpool.tile([128, ND, BL], bf16)     # residual stream, bf16
        x0r = xpool.tile([128, ND, BL], bf16)     # original x (mult operand)
        bf16_unused = None
        with tc.tile_pool(name="xnat_p", bufs=1) as xnat_p:
            xnat = xnat_p.tile([128, NBT, D], f32)
            # first batch-group arrives in column chunks so the dc=0
            # transposes can start after 256KB instead of 1MB
            for dc in range(ND):
                for t in range(4):
                    nc.sync.dma_start(
                        out=xnat[:, t, 128 * dc:128 * (dc + 1)],
                        in_=x_d[128 * t:128 * (t + 1),
                                128 * dc:128 * (dc + 1)])
                if dc == 0:
                    prep_V0([0, 1])
                elif dc == 1:
                    prep_C(0)
                elif dc == 2:
                    prep_V0([2, 3])
            for t in range(4, NBT):
                nc.sync.dma_start(
                    out=xnat[:, t, :],
                    in_=x_d[128 * t:128 * (t + 1), :])
            for g in range(NBT // 4):
                for dc in range(ND):
                    pxt = ptr.tile([128, 512], f32, name=f"pxt{dc}_{g}",
                                   tag="tr")
                    for i in range(4):
                        nc.tensor.transpose(
                            pxt[:, 128 * i:128 * (i + 1)],
                            xnat[:, 4 * g + i, 128 * dc:128 * (dc + 1)],
                            ident)
                    nc.any.tensor_copy(
                        xlT[:, dc, 512 * g:512 * (g + 1)], pxt)
                    nc.any.tensor_copy(
                        x0r[:, dc, 512 * g:512 * (g + 1)], pxt)
                emit_split(0, g)
                if g == 0:
                    prep_U(0)
                elif g == 1:
                    prep_V(1)
                    prep_C(1)
                elif g == 2:
                    prep_U(1)
                elif g == 3:
                    prep_V(2)
                    prep_C(2)
                    prep_U(2)

        # ---- main layer loop ----
        for l in range(L):
            for c in range(NBC):
                cols = slice(512 * c, 512 * (c + 1))
                x8h, x8l = x8tiles.pop((l, c))

                zr = []
                for e in range(E):
                    ph = ph_p.tile([128, 512], f32, name=f"ph{l}_{c}_{e}",
                                   tag="ph")
                    k = 0
                    for (W, X) in ((V8h, x8h), (V8l, x8h), (V8h, x8l)):
                        for kdp in range(2):
                            nc.tensor.matmul(
                                ph,
                                lhsT=W[:, l, e, 2 * kdp:2 * kdp + 2, :],
                                rhs=X[:, 2 * kdp:2 * kdp + 2, :],
                                start=(k == 0), stop=(k == 5),
                                perf_mode=DRmode)
                            k += 1
                    hr = hz_p.tile([128, 512], mmdt, name=f"h{l}_{c}_{e}",
                                   tag="h", bufs=6)
                    nc.scalar.activation(hr, ph, Tanh, scale=1.0 / WSCALE)

                    pz = pz_p.tile([128, 512], f32, name=f"pz{l}_{c}_{e}",
                                   tag="pz")
                    nc.tensor.matmul(pz, lhsT=Cr[:, l, e, :], rhs=hr,
                                     start=True, stop=True)
                    z = hz_p.tile([128, 512], mmdt, name=f"z{l}_{c}_{e}",
                                  tag="z", bufs=7 if has_bias else 8)
                    nc.scalar.activation(z, pz, Tanh)
                    zr.append(z)

                for dc in range(ND):
                    pu = pu_p.tile([128, 512], f32, name=f"pu{l}_{c}_{dc}",
                                   tag="pu")
                    for e in range(E):
                        nc.tensor.matmul(
                            pu, lhsT=Ur[:, l, e, dc, :], rhs=zr[e],
                            start=(e == 0),
                            stop=(e == E - 1 and not has_bias))
                    if has_bias:
                        nc.tensor.matmul(
                            pu,
                            lhsT=bias4[:, l * D + 128 * dc:l * D + 128 * (dc + 1)],
                            rhs=ones_r, start=False, stop=True)
                    tmp = tmp_p.tile([128, 512], bf16, name=f"tmp{l}_{c}_{dc}",
                                     tag="tmp")
                    nc.vector.tensor_mul(
                        tmp, pu, x0r[:, dc, cols])
                    nc.vector.tensor_add(
                        xlT[:, dc, cols], xlT[:, dc, cols], tmp)
                if l + 1 < L:
                    emit_split(l + 1, c)

                if l == L - 1:
                    # store this chunk: transpose back to natural + DMA out
                    for t in range(4 * c, 4 * (c + 1)):
                        pot = ptr.tile([128, 512], bf16, name=f"pot{t}",
                                       tag="tr")
                        for dc in range(ND):
                            nc.tensor.transpose(
                                pot[:, 128 * dc:128 * (dc + 1)],
                                xlT[:, dc, 128 * t:128 * (t + 1)], identb)
                        onat = onat_p.tile([128, D], f32, name=f"onat{t}",
                                           tag="onat")
                        nc.any.tensor_copy(onat, pot)
                        nc.sync.dma_start(
                            out=out_d[128 * t:128 * (t + 1), :], in_=onat)


    nc.finalize()
    return nc


def _get_prog(has_bias: bool, use_f32r: bool = True):
    key = (has_bias, use_f32r)
    if key not in _prog_cache:
        _prog_cache[key] = _build(has_bias, use_f32r)
    return _prog_cache[key]


def _run(inputs, trace=False, use_f32r=True):
    x = np.ascontiguousarray(np.asarray(inputs["x"], dtype=np.float32))
    Us = np.ascontiguousarray(np.asarray(inputs["Us"], dtype=np.float32))
    Cs = np.ascontiguousarray(np.asarray(inputs["Cs"], dtype=np.float32))
    Vs = np.ascontiguousarray(np.asarray(inputs["Vs"], dtype=np.float32))
    b = np.ascontiguousarray(np.asarray(inputs["b"], dtype=np.float32))
    assert x.shape == (B, D), x.shape
    has_bias = bool(np.any(b))
    nc = _get_prog(has_bias, use_f32r)
    shards = np.split(x, NCORES, axis=0)
    in_maps = []
    for i in range(NCORES):
        m = {"x": shards[i], "Us": Us, "Cs": Cs, "Vs": Vs}
        if has_bias:
            m["b"] = b
        in_maps.append(m)
    res = run_bass_kernel_spmd(nc, in_maps, core_ids=list(range(NCORES)),
                               trace=trace)
    out = np.concatenate([res.results[i]["out"] for i in range(NCORES)],
                         axis=0)
    return out, res


def kernel(**inputs) -> np.ndarray:
    out, _ = _run(inputs)
    return out

